# revision 7
# baseline (speedup 1.0000x reference)
"""HSTU block preprocessor as a Trainium2 Bass kernel.

The op is a pure row permutation:
  - interleave item/action rows: I[2i] = item[i], I[2i+1] = action[i]
  - per-sample output: [ctx1_b ; ctx2_b ; interleaved item/action of sample b]

Strategy: data-parallel over the batch dim.  Samples are split into 8
contiguous ranges balanced by output bytes.  One shared SPMD program is
compiled per call (offsets are known then), with each core's fully static
DRAM->DRAM DMA descriptor list selected by an O(1) Switch on partition_id.
"""

import numpy as np

N_CORES = 8


def _split_samples(out_offsets, n_cores):
    """Contiguous sample ranges with ~equal output rows per core."""
    B = len(out_offsets) - 1
    total = int(out_offsets[-1])
    bounds = [0]
    for c in range(1, n_cores):
        t = total * c / n_cores
        i = int(np.searchsorted(out_offsets, t, side="left"))
        bounds.append(min(max(i, bounds[-1]), B))
    bounds.append(B)
    return bounds


def _build_plans(io, c1, c2, oo, bounds):
    """Per-core list of (dst_row, dst_step, src_name, src_row, n_rows)."""
    il = np.diff(io)
    l1 = np.diff(c1)
    l2 = np.diff(c2)
    plans = []
    for c in range(N_CORES):
        b0, b1 = bounds[c], bounds[c + 1]
        base_i, base_1, base_2, base_o = io[b0], c1[b0], c2[b0], oo[b0]
        plan = []
        for b in range(b0, b1):
            d0 = int(oo[b] - base_o)
            if l1[b] > 0:
                plan.append((d0, 1, "ctx1", int(c1[b] - base_1), int(l1[b])))
            if l2[b] > 0:
                plan.append((d0 + int(l1[b]), 1, "ctx2", int(c2[b] - base_2), int(l2[b])))
            di = d0 + int(l1[b] + l2[b])
            if il[b] > 0:
                plan.append((di, 2, "item", int(io[b] - base_i), int(il[b])))
                plan.append((di + 1, 2, "action", int(io[b] - base_i), int(il[b])))
        plans.append(plan)
    return plans


def _prepare(item_values, action_values, ctx1_values, ctx2_values,
             item_offsets, ctx1_offsets, ctx2_offsets, repeat_loop=False):
    """Build (nc, in_maps, meta). With repeat_loop, the program takes an extra
    [1,1] uint32 "reps" input and executes the whole DMA plan that many times
    (for wall-clock delta benchmarking)."""
    import concourse.bass as bass
    import concourse.mybir as mybir

    item_values = np.ascontiguousarray(np.asarray(item_values), dtype=np.float32)
    action_values = np.ascontiguousarray(np.asarray(action_values), dtype=np.float32)
    ctx1_values = np.ascontiguousarray(np.asarray(ctx1_values), dtype=np.float32)
    ctx2_values = np.ascontiguousarray(np.asarray(ctx2_values), dtype=np.float32)
    io = np.asarray(item_offsets).astype(np.int64)
    c1 = np.asarray(ctx1_offsets).astype(np.int64)
    c2 = np.asarray(ctx2_offsets).astype(np.int64)

    B = io.shape[0] - 1
    D = int(item_values.shape[1])
    il = np.diff(io)
    l1 = np.diff(c1)
    l2 = np.diff(c2)
    out_len = l1 + l2 + 2 * il
    oo = np.zeros(B + 1, np.int64)
    oo[1:] = np.cumsum(out_len)
    total_out = int(oo[-1])

    bounds = _split_samples(oo, N_CORES)
    plans = _build_plans(io, c1, c2, oo, bounds)

    def rows(off):
        return [int(off[bounds[c + 1]] - off[bounds[c]]) for c in range(N_CORES)]

    item_rows = rows(io)
    out_rows = rows(oo)
    ITEM_PAD = max(max(item_rows), 1)
    CTX1_PAD = max(max(rows(c1)), 1)
    CTX2_PAD = max(max(rows(c2)), 1)
    OUT_PAD = max(max(out_rows), 1)

    in_maps = []
    for c in range(N_CORES):
        b0, b1 = bounds[c], bounds[c + 1]
        m = {}
        for name, full, off, pad in (
            ("item", item_values, io, ITEM_PAD),
            ("action", action_values, io, ITEM_PAD),
            ("ctx1", ctx1_values, c1, CTX1_PAD),
            ("ctx2", ctx2_values, c2, CTX2_PAD),
        ):
            buf = np.zeros((pad, D), np.float32)
            r0, r1 = int(off[b0]), int(off[b1])
            buf[: r1 - r0] = full[r0:r1]
            m[name] = buf
        if repeat_loop:
            m["reps"] = np.array([[1]], dtype=np.uint32)
        in_maps.append(m)

    nc = bass.Bass()
    f32 = mybir.dt.float32
    t = {
        "item": nc.declare_dram_parameter("item", [ITEM_PAD, D], f32, isOutput=False),
        "action": nc.declare_dram_parameter("action", [ITEM_PAD, D], f32, isOutput=False),
        "ctx1": nc.declare_dram_parameter("ctx1", [CTX1_PAD, D], f32, isOutput=False),
        "ctx2": nc.declare_dram_parameter("ctx2", [CTX2_PAD, D], f32, isOutput=False),
    }
    t_reps = (
        nc.declare_dram_parameter("reps", [1, 1], mybir.dt.uint32, isOutput=False)
        if repeat_loop
        else None
    )
    t_out = nc.declare_dram_parameter("out", [OUT_PAD, D], f32, isOutput=True)

    with nc.Block() as block, nc.semaphore("dma_sem") as dma_sem:

        @block.sync
        def _(sync):
            pid = sync.partition_id()
            reps = sync.value_load(t_reps[0:1, 0:1]) if repeat_loop else None

            def emit(plan):
                cnt = 0
                for dst, step, name, src, n in plan:
                    src_ap = t[name][src : src + n, :]
                    dst_ap = t_out[dst : dst + step * (n - 1) + 1 : step, :]
                    sync.dma_start(out=dst_ap, in_=src_ap).then_inc(dma_sem, 16)
                    cnt += 1
                if cnt:
                    sync.wait_ge(dma_sem, 16 * cnt)
                return cnt

            for c in sync.Switch(pid, N_CORES):
                if repeat_loop:
                    with sync.Fori(0, reps, 1):
                        if emit(plans[c]):
                            sync.sem_clear(dma_sem)
                else:
                    emit(plans[c])

    meta = dict(bounds=bounds, oo=oo, total_out=total_out, D=D,
                out_len=out_len, out_rows=out_rows)
    return nc, in_maps, meta


def _assemble(res_list, meta):
    merged_values = np.empty((meta["total_out"], meta["D"]), np.float32)
    oo, bounds = meta["oo"], meta["bounds"]
    for c in range(N_CORES):
        r0, r1 = int(oo[bounds[c]]), int(oo[bounds[c + 1]])
        merged_values[r0:r1] = res_list[c]["out"][: r1 - r0]
    return merged_values, meta["out_len"].astype(np.int32)


def _run(*args, trace=False, trace_cores=None, **kw):
    from concourse.bass_utils import run_bass_kernel_spmd

    nc, in_maps, meta = _prepare(*args, **kw)
    res = run_bass_kernel_spmd(nc, in_maps, core_ids=list(range(N_CORES)),
                               trace=trace, trace_cores=trace_cores)
    merged_values, merged_lengths = _assemble(res.results, meta)
    return merged_values, merged_lengths, res


def kernel(item_values, action_values, ctx1_values, ctx2_values,
           item_offsets, ctx1_offsets, ctx2_offsets):
    return _run(item_values, action_values, ctx1_values, ctx2_values,
                item_offsets, ctx1_offsets, ctx2_offsets)[:2]


# revision 13
# speedup vs baseline: 1.0119x; 1.0119x over previous
"""HSTU block preprocessor as a Trainium2 Bass kernel.

The op is a pure row permutation:
  - interleave item/action rows: I[2i] = item[i], I[2i+1] = action[i]
  - per-sample output: [ctx1_b ; ctx2_b ; interleaved item/action of sample b]

Strategy: data-parallel over the batch dim.  Samples are split into 8
contiguous ranges balanced by moved bytes.  One shared SPMD program is
compiled per call (offsets are known then); each core's fully static
DRAM->DRAM DMA descriptor list is selected by an O(1) Switch on partition_id.
Bulk transfers are split across both HWDGE rings: item rows (stride-2 dst)
plus ctx chunks on the SP ring, action rows (stride-2 dst) on the ACT ring.
"""

import numpy as np

N_CORES = 8


def _split_samples(weights, n_cores):
    """Contiguous sample ranges with ~equal cumulative weight per core."""
    B = len(weights)
    cum = np.zeros(B + 1, np.float64)
    cum[1:] = np.cumsum(weights)
    bounds = [0]
    for c in range(1, n_cores):
        t = cum[-1] * c / n_cores
        i = int(np.searchsorted(cum, t, side="left"))
        bounds.append(min(max(i, bounds[-1]), B))
    bounds.append(B)
    return bounds


def _build_plans(io, c1, c2, oo, bounds):
    """Per-core, per-ring lists of (dst_row, dst_step, src_name, src_row, n)."""
    il = np.diff(io)
    l1 = np.diff(c1)
    l2 = np.diff(c2)
    plans = []
    for c in range(N_CORES):
        b0, b1 = bounds[c], bounds[c + 1]
        base_i, base_1, base_2, base_o = io[b0], c1[b0], c2[b0], oo[b0]
        sp, act = [], []
        for b in range(b0, b1):
            d0 = int(oo[b] - base_o)
            if l1[b] > 0:
                sp.append((d0, 1, "ctx1", int(c1[b] - base_1), int(l1[b])))
            if l2[b] > 0:
                sp.append((d0 + int(l1[b]), 1, "ctx2", int(c2[b] - base_2), int(l2[b])))
            di = d0 + int(l1[b] + l2[b])
            if il[b] > 0:
                sp.append((di, 2, "item", int(io[b] - base_i), int(il[b])))
                act.append((di + 1, 2, "action", int(io[b] - base_i), int(il[b])))
        # ctx chunks after bulk item chunks so small transfers trail
        sp.sort(key=lambda e: (e[1] == 1, e[0]))
        plans.append((sp, act))
    return plans


def _prepare(item_values, action_values, ctx1_values, ctx2_values,
             item_offsets, ctx1_offsets, ctx2_offsets, repeat_loop=False):
    import concourse.bass as bass
    import concourse.mybir as mybir

    item_values = np.ascontiguousarray(np.asarray(item_values), dtype=np.float32)
    action_values = np.ascontiguousarray(np.asarray(action_values), dtype=np.float32)
    ctx1_values = np.ascontiguousarray(np.asarray(ctx1_values), dtype=np.float32)
    ctx2_values = np.ascontiguousarray(np.asarray(ctx2_values), dtype=np.float32)
    io = np.asarray(item_offsets).astype(np.int64)
    c1 = np.asarray(ctx1_offsets).astype(np.int64)
    c2 = np.asarray(ctx2_offsets).astype(np.int64)

    B = io.shape[0] - 1
    D = int(item_values.shape[1])
    il = np.diff(io)
    l1 = np.diff(c1)
    l2 = np.diff(c2)
    out_len = l1 + l2 + 2 * il
    oo = np.zeros(B + 1, np.int64)
    oo[1:] = np.cumsum(out_len)
    total_out = int(oo[-1])

    bounds = _split_samples(2 * il + l1 + l2, N_CORES)
    plans = _build_plans(io, c1, c2, oo, bounds)

    def rows(off):
        return [int(off[bounds[c + 1]] - off[bounds[c]]) for c in range(N_CORES)]

    out_rows = rows(oo)
    ITEM_PAD = max(max(rows(io)), 1)
    CTX1_PAD = max(max(rows(c1)), 1)
    CTX2_PAD = max(max(rows(c2)), 1)
    OUT_PAD = max(max(out_rows), 1)

    in_maps = []
    for c in range(N_CORES):
        b0, b1 = bounds[c], bounds[c + 1]
        m = {}
        for name, full, off, pad in (
            ("item", item_values, io, ITEM_PAD),
            ("action", action_values, io, ITEM_PAD),
            ("ctx1", ctx1_values, c1, CTX1_PAD),
            ("ctx2", ctx2_values, c2, CTX2_PAD),
        ):
            buf = np.zeros((pad, D), np.float32)
            r0, r1 = int(off[b0]), int(off[b1])
            buf[: r1 - r0] = full[r0:r1]
            m[name] = buf
        if repeat_loop:
            m["reps"] = np.array([[1]], dtype=np.uint32)
        in_maps.append(m)

    nc = bass.Bass()
    f32 = mybir.dt.float32
    t = {
        "item": nc.declare_dram_parameter("item", [ITEM_PAD, D], f32, isOutput=False),
        "action": nc.declare_dram_parameter("action", [ITEM_PAD, D], f32, isOutput=False),
        "ctx1": nc.declare_dram_parameter("ctx1", [CTX1_PAD, D], f32, isOutput=False),
        "ctx2": nc.declare_dram_parameter("ctx2", [CTX2_PAD, D], f32, isOutput=False),
    }
    t_reps = (
        nc.declare_dram_parameter("reps", [1, 1], mybir.dt.uint32, isOutput=False)
        if repeat_loop
        else None
    )
    t_out = nc.declare_dram_parameter("out", [OUT_PAD, D], f32, isOutput=True)

    with (
        nc.Block() as block,
        nc.semaphore("sp_sem") as sp_sem,
        nc.semaphore("act_sem") as act_sem,
    ):

        def emit(eng, plan, sem, it):
            cnt = len(plan)
            for dst, step, name, src, n in plan:
                src_ap = t[name][src : src + n, :]
                dst_ap = t_out[dst : dst + step * (n - 1) + 1 : step, :]
                eng.dma_start(out=dst_ap, in_=src_ap).then_inc(sem, 16)
            if cnt:
                tgt = 16 * cnt if it is None else (it + 1) * (16 * cnt)
                eng.wait_ge(sem, tgt)

        def engine_stream(eng, ring, sem):
            reps = eng.value_load(t_reps[0:1, 0:1]) if repeat_loop else None
            pid = eng.partition_id()
            for c in eng.Switch(pid, N_CORES):
                plan = plans[c][ring]
                if repeat_loop:
                    with eng.Fori(0, reps, 1) as i:
                        emit(eng, plan, sem, i)
                else:
                    emit(eng, plan, sem, None)

        @block.sync
        def _(sync):
            engine_stream(sync, 0, sp_sem)

        @block.scalar
        def _(scalar):
            engine_stream(scalar, 1, act_sem)

    meta = dict(bounds=bounds, oo=oo, total_out=total_out, D=D,
                out_len=out_len, out_rows=out_rows)
    return nc, in_maps, meta


def _assemble(res_list, meta):
    merged_values = np.empty((meta["total_out"], meta["D"]), np.float32)
    oo, bounds = meta["oo"], meta["bounds"]
    for c in range(N_CORES):
        r0, r1 = int(oo[bounds[c]]), int(oo[bounds[c + 1]])
        merged_values[r0:r1] = res_list[c]["out"][: r1 - r0]
    return merged_values, meta["out_len"].astype(np.int32)


def _run(*args, trace=False, trace_cores=None, **kw):
    from concourse.bass_utils import run_bass_kernel_spmd

    nc, in_maps, meta = _prepare(*args, **kw)
    res = run_bass_kernel_spmd(nc, in_maps, core_ids=list(range(N_CORES)),
                               trace=trace, trace_cores=trace_cores)
    merged_values, merged_lengths = _assemble(res.results, meta)
    return merged_values, merged_lengths, res


def kernel(item_values, action_values, ctx1_values, ctx2_values,
           item_offsets, ctx1_offsets, ctx2_offsets):
    return _run(item_values, action_values, ctx1_values, ctx2_values,
                item_offsets, ctx1_offsets, ctx2_offsets)[:2]
